# revision 58
# baseline (speedup 1.0000x reference)
"""GAT-style attention head (gnn_message_passing) on 8 Trainium2 NeuronCores.

Math (per node b with N=32 neighbors, D=U=128):
    c_n = W @ a_neigh ; c_s = W @ a_self                  (tiny, host)
    nf1[b,n] = neigh[b,n,:] . c_n
    sf1[b]   = self[b,:] . c_s
    e        = exp(lrelu(sf1[b] + nf1[b,n]))
    agg[b,:] = sum_n e[b,n] * neigh[b,n,:]
    Z[b]     = sum_n e[b,n]
    out[b,:] = lrelu((agg[b,:] @ W) / Z[b])

Device-side trick set (vs the f32 baseline, 1.28ms -> ~0.19ms):
  * bf16 inputs in HBM (tolerance is 2e-2): halves HBM traffic -> DMA
    roofline halves to ~155us/core.
  * Host pre-scales neigh rows by c_n elementwise and self rows by c_s, and
    the device projection uses W' = W / c_n[:, None].  Then
      nf1 = row-sum(neigh'),  sf1 = row-sum(self'),  agg' @ W' == agg @ W
    exactly, so the big per-supertile elementwise multiply disappears.
  * Host repacks neigh into supertile-major order so each supertile load is
    one DMA of 128 partitions x 8KB contiguous (line rate).
  * nf1 row-sums as a binary tree of bf16 tensor_tensor adds (these pack at
    2 elem/cycle on the DVE; tensor_reduce does not pack) down to 8 partials,
    then one small f32 tensor_reduce.
  * Supertile loads alternate between the two HWDGE rings (nc.sync /
    nc.scalar) - each ring serializes its own DMAs, alternating keeps two
    loads in flight and the SDMA engines saturated.
  * sf1 broadcast to (row, tile) layout via a tiny mask matmul on the PE
    (replaces a DRAM roundtrip).
  * e4 block-diagonal built by one GpSimd tensor_tensor with a stride-0
    broadcast AP (replaces 4 small scatter DMAs).
  * lrelu via ACT Prelu (same activation table as Exp/Copy -> no table
    reloads); final scale-by-1/Z fused into the last Prelu.
  * Epilogue (1/Z, projection, output write) software-pipelined one
    iteration late so the in-order DVE queue never waits on the PE's Z.

Sharding: batch dim across 8 cores (6250 nodes/core, padded to 6272 = 49
supertiles of 128 nodes; a supertile = 32 tiles of 128 (b,n)-rows = 4 nodes).
"""

import numpy as np
import ml_dtypes

B, N, D = 50000, 32, 128
NCORES = 8
NODES_PER_CORE = B // NCORES            # 6250
SUPER = 128                              # nodes per supertile
NS = (NODES_PER_CORE + SUPER - 1) // SUPER   # 49 supertiles
NODES_PAD = NS * SUPER                   # 6272
ROWS_SUPER = SUPER * N                   # 4096 (b,n)-rows per supertile
NTILES = ROWS_SUPER // 128               # 32 tiles per supertile
ALPHA = 0.2
BF16 = ml_dtypes.bfloat16


def build_program(ns=NS):
    from concourse import mybir
    from concourse.bacc import Bacc
    from concourse.bass import ds
    from concourse.tile import TileContext

    f32 = mybir.dt.float32
    bf16 = mybir.dt.bfloat16
    nc = Bacc()

    neigh = nc.declare_dram_parameter("neigh", [ns * 128, NTILES * D], bf16, isOutput=False)
    selfv = nc.declare_dram_parameter("selfv", [128, ns * D], bf16, isOutput=False)
    wp_in = nc.declare_dram_parameter("wprime", [D, D], bf16, isOutput=False)
    selmask_in = nc.declare_dram_parameter("selmask", [128, 128], bf16, isOutput=False)
    mask4_in = nc.declare_dram_parameter("mask4", [128, NTILES], bf16, isOutput=False)
    prep_in = nc.declare_dram_parameter("prep", [128, 256], bf16, isOutput=False)
    ones1_in = nc.declare_dram_parameter("ones1", [128, 1], bf16, isOutput=False)
    out_d = nc.declare_dram_parameter("out", [128, ns * D], bf16, isOutput=True)

    add = mybir.AluOpType.add
    mult = mybir.AluOpType.mult
    AF = mybir.ActivationFunctionType

    with TileContext(nc) as tc:
        with (
            tc.tile_pool(name="consts", bufs=1) as cpool,
            tc.tile_pool(name="x", bufs=4) as xpool,
            tc.tile_pool(name="work", bufs=3) as wpool,
            tc.tile_pool(name="small", bufs=6) as spool,
            tc.tile_pool(name="psA", bufs=2, space="PSUM") as psA,
            tc.tile_pool(name="psB", bufs=2, space="PSUM") as psB,
            tc.tile_pool(name="psZ", bufs=2, space="PSUM") as psZ,
            tc.tile_pool(name="psS", bufs=2, space="PSUM") as psS,
        ):
            w_sb = cpool.tile([128, D], bf16, tag="w")
            selmask = cpool.tile([128, 128], bf16, tag="selmask")
            mask4 = cpool.tile([128, NTILES], bf16, tag="mask4")
            prep = cpool.tile([128, 256], bf16, tag="prep")
            ones1 = cpool.tile([128, 1], bf16, tag="ones1")
            self_sb = cpool.tile([128, ns * D], bf16, tag="selfsb")
            out_acc = cpool.tile([128, ns * D], bf16, tag="outacc")
            # selfv leads the sync ring (sf1 chain needs it first); the
            # small consts ride the scalar ring so x[1] still lands early.
            nc.sync.dma_start(out=self_sb, in_=selfv[:, :])
            nc.scalar.dma_start(out=w_sb, in_=wp_in[:, :])
            nc.scalar.dma_start(out=selmask, in_=selmask_in[:, :])
            nc.scalar.dma_start(out=mask4, in_=mask4_in[:, :])
            nc.scalar.dma_start(out=prep, in_=prep_in[:, :])
            nc.scalar.dma_start(out=ones1, in_=ones1_in[:, :])

            # sf1 for all supertiles: two bf16 tree levels (2 elem/cycle)
            # then one f32 reduce: [128, ns]
            sf1_all = cpool.tile([128, ns], f32, tag="sf1all")
            with nc.allow_low_precision("bf16 tree partials, f32 final"):
                sft1 = cpool.tile([128, ns * 64], bf16, tag="sft1")
                sview = self_sb.rearrange("p (s2 d) -> p s2 d", s2=ns)
                nc.vector.tensor_tensor(
                    sft1.rearrange("p (s2 d) -> p s2 d", s2=ns),
                    sview[:, :, ds(0, 64)], sview[:, :, ds(64, 64)], op=add,
                )
                sft2 = cpool.tile([128, ns * 32], bf16, tag="sft2")
                t1v = sft1.rearrange("p (s2 d) -> p s2 d", s2=ns)
                nc.vector.tensor_tensor(
                    sft2.rearrange("p (s2 d) -> p s2 d", s2=ns),
                    t1v[:, :, ds(0, 32)], t1v[:, :, ds(32, 32)], op=add,
                )
            nc.vector.tensor_reduce(
                sf1_all,
                sft2.rearrange("p (s2 d) -> p s2 d", s2=ns),
                mybir.AxisListType.X,
                add,
            )

            prev = None
            # 256-node super-iterations (two 128-node supertiles fused) to
            # halve per-op instruction overheads; one 128-node tail.
            iters = [(2 * k, 2) for k in range((ns - 1) // 2)] + [(ns - 1, 1)]
            for s0, nb in iters:
                T = NTILES * nb
                # ---- load nb supertiles: per partition nb x 8KB chunks ----
                x = xpool.tile([128, 2 * NTILES * D], bf16, tag="x")
                dma_eng = nc.sync if s0 % 4 == 0 else nc.scalar
                xin = neigh[ds(128 * s0, 128 * nb), :]
                xout = x[:, ds(0, T * D)]
                if nb == 2:
                    xin = xin.rearrange("(j p) d -> p j d", p=128)
                    xout = xout.rearrange("p (j d) -> p j d", j=2)
                dma_eng.dma_start(out=xout, in_=xin)

                # ---- nf1 tree (bf16 TT adds pack 2 elem/cycle) ----
                lvl_in = x
                w = D
                with nc.allow_low_precision("bf16 tree partials, f32 final"):
                    while w > 8:
                        h = w // 2
                        nxt = wpool.tile([128, 2 * NTILES * h], bf16, tag=f"tree{h}")
                        nc.vector.tensor_tensor(
                            nxt[:, ds(0, T * h)].rearrange("p (i d) -> p i d", i=T),
                            lvl_in[:, ds(0, T * w)].rearrange(
                                "p (i d) -> p i d", i=T
                            )[:, :, ds(0, h)],
                            lvl_in[:, ds(0, T * w)].rearrange(
                                "p (i d) -> p i d", i=T
                            )[:, :, ds(h, h)],
                            op=add,
                        )
                        lvl_in = nxt
                        w = h
                F1 = spool.tile([128, 2 * NTILES], f32, tag="F1")
                nc.vector.tensor_reduce(
                    F1[:, ds(0, T)],
                    lvl_in[:, ds(0, T * 8)].rearrange("p (i m) -> p i m", m=8),
                    mybir.AxisListType.X,
                    add,
                )

                # ---- sf1e via mask matmuls, one per 128-node half ----
                rhsv = spool.tile([128, 2 * NTILES], bf16, tag="rhsv")
                sf1e_ps = psS.tile([128, 2 * NTILES], f32, tag="sf1e")
                for j in range(nb):
                    nc.scalar.activation(
                        rhsv[:, ds(NTILES * j, NTILES)], mask4, AF.Copy,
                        scale=sf1_all[:, ds(s0 + j, 1)],
                    )
                    nc.tensor.matmul(
                        sf1e_ps[:, ds(NTILES * j, NTILES)],
                        selmask, rhsv[:, ds(NTILES * j, NTILES)],
                        start=True, stop=True,
                    )

                # ---- logits -> lrelu -> e ----
                lk = spool.tile([128, 2 * NTILES], f32, tag="lk")
                nc.vector.tensor_tensor(
                    lk[:, ds(0, T)], F1[:, ds(0, T)], sf1e_ps[:, ds(0, T)], op=add
                )
                lr = spool.tile([128, 2 * NTILES], f32, tag="lr")
                nc.scalar.activation(
                    lr[:, ds(0, T)], lk[:, ds(0, T)], AF.Prelu, alpha=ALPHA
                )
                e_t = spool.tile([128, 2 * NTILES], bf16, tag="e")
                nc.scalar.activation(e_t[:, ds(0, T)], lr[:, ds(0, T)], AF.Exp)

                # ---- e4 block-diagonal ----
                e4 = wpool.tile([128, 2 * NTILES * 4], bf16, tag="e4")
                e_bc = e_t[:, ds(0, T)].rearrange(
                    "p (i o) -> p i o", o=1
                ).broadcast_to((128, T, 4))
                nc.gpsimd.tensor_tensor(
                    e4[:, ds(0, 4 * T)].rearrange("p (i g) -> p i g", g=4),
                    prep[:, ds(0, 4 * T)].rearrange("p (i g) -> p i g", g=4),
                    e_bc,
                    op=mult,
                )

                # ---- Z (one matmul per half) then aggregation on the PE ----
                zs_ps = psZ.tile([128, 2], f32, tag="z")
                for j in range(nb):
                    nc.tensor.matmul(
                        zs_ps[:, ds(j, 1)],
                        e4[:, ds(128 * j, 128)], ones1,
                        start=True, stop=True,
                    )
                aggT_ps = psA.tile([128, 256], f32, tag="aggT")
                for i in range(T):
                    nc.tensor.matmul(
                        aggT_ps[:, ds(4 * i, 4)],
                        x[:, ds(D * i, D)],
                        e4[:, ds(4 * i, 4)],
                        start=True, stop=True,
                    )

                # ---- epilogue for the PREVIOUS super-iteration ----
                if prev is not None:
                    p_s0, p_nb, p_z, p_agg = prev
                    zinv = spool.tile([128, 2], f32, tag="zinv")
                    nc.vector.reciprocal(
                        zinv[:, ds(0, p_nb)], p_z[:, ds(0, p_nb)]
                    )
                    aggT_sb = wpool.tile([128, 256], bf16, tag="aggTsb")
                    nc.scalar.copy(
                        aggT_sb[:, ds(0, 128 * p_nb)], p_agg[:, ds(0, 128 * p_nb)]
                    )
                    out_ps = psB.tile([128, 256], f32, tag="outps")
                    for j in range(p_nb):
                        nc.tensor.matmul(
                            out_ps[:, ds(128 * j, 128)],
                            aggT_sb[:, ds(128 * j, 128)], w_sb,
                            start=True, stop=True,
                        )
                        nc.scalar.activation(
                            out_acc[:, ds(D * (p_s0 + j), D)],
                            out_ps[:, ds(128 * j, 128)], AF.Prelu,
                            scale=zinv[:, ds(j, 1)], alpha=ALPHA,
                        )
                prev = (s0, nb, zs_ps, aggT_ps)

            # drain the last super-iteration's epilogue
            p_s0, p_nb, p_z, p_agg = prev
            zinv = spool.tile([128, 2], f32, tag="zinv")
            nc.vector.reciprocal(zinv[:, ds(0, p_nb)], p_z[:, ds(0, p_nb)])
            aggT_sb = wpool.tile([128, 256], bf16, tag="aggTsb")
            nc.scalar.copy(
                aggT_sb[:, ds(0, 128 * p_nb)], p_agg[:, ds(0, 128 * p_nb)]
            )
            out_ps = psB.tile([128, 256], f32, tag="outps")
            for j in range(p_nb):
                nc.tensor.matmul(
                    out_ps[:, ds(128 * j, 128)],
                    aggT_sb[:, ds(128 * j, 128)], w_sb,
                    start=True, stop=True,
                )
                nc.scalar.activation(
                    out_acc[:, ds(D * (p_s0 + j), D)],
                    out_ps[:, ds(128 * j, 128)], AF.Prelu,
                    scale=zinv[:, ds(j, 1)], alpha=ALPHA,
                )

            nc.sync.dma_start(out=out_d[:, :], in_=out_acc)

    nc.compile()
    return nc


_CACHE = {}


def _get_program():
    if "nc" not in _CACHE:
        _CACHE["nc"] = build_program()
    return _CACHE["nc"]


def make_consts(W, a_self, a_neigh):
    """Host-side parameter prep: c_n/c_s projections and mask constants."""
    W = np.ascontiguousarray(W, dtype=np.float32)
    c_n = (W @ np.asarray(a_neigh, dtype=np.float32)[:, 0]).astype(np.float32)
    c_s = (W @ np.asarray(a_self, dtype=np.float32)[:, 0]).astype(np.float32)
    # guard against pathological zeros (probability ~0 for random W)
    c_n_safe = np.where(np.abs(c_n) < 1e-30, 1e-30, c_n).astype(np.float32)
    wprime = (W / c_n_safe[:, None]).astype(BF16)

    p = np.arange(128)
    i = np.arange(NTILES)
    selmask = (p[:, None] % 4 == (np.arange(128)[None, :] // 32)).astype(BF16)
    mask4 = (p[:, None] // 4 == i[None, :]).astype(BF16)
    prep = ((p[:, None] // 32) == (np.arange(256)[None, :] % 4)).astype(BF16)
    ones1 = np.ones((128, 1), dtype=BF16)
    consts = {
        "wprime": wprime,
        "selmask": selmask,
        "mask4": mask4,
        "prep": prep,
        "ones1": ones1,
    }
    return consts, c_n, c_s


def build_in_maps(self_vecs, neigh_vecs, W, a_self, a_neigh):
    """Shard + pre-scale + repack all inputs into per-core DRAM images."""
    consts, c_n, c_s = make_consts(W, a_self, a_neigh)

    neigh_vecs = np.asarray(neigh_vecs, dtype=np.float32)
    self_vecs = np.asarray(self_vecs, dtype=np.float32)
    # pre-scale (folded back exactly by W' = W / c_n on-device)
    xs_all = (neigh_vecs * c_n[None, None, :]).astype(BF16)     # [B, N, D]
    ss_all = (self_vecs * c_s[None, :]).astype(BF16)            # [B, D]

    in_maps = []
    for k in range(NCORES):
        lo = k * NODES_PER_CORE
        hi = lo + NODES_PER_CORE
        xs = np.zeros((NODES_PAD, N, D), dtype=BF16)
        xs[:NODES_PER_CORE] = xs_all[lo:hi]
        # rows r = 32*b_local + n ; tile i = rows 128i..128i+127 ; partition p
        # hbuf[s, p, i, d] = xs_rows[s, 128*i + p, d]
        hbuf = (
            xs.reshape(NS, SUPER * N, D)
            .reshape(NS, NTILES, 128, D)
            .transpose(0, 2, 1, 3)
            .reshape(NS * 128, NTILES * D)
        )
        hbuf = np.ascontiguousarray(hbuf)

        ss = np.zeros((NODES_PAD, D), dtype=BF16)
        ss[:NODES_PER_CORE] = ss_all[lo:hi]
        sbuf = np.ascontiguousarray(
            ss.reshape(NS, 128, D).transpose(1, 0, 2).reshape(128, NS * D)
        )
        in_maps.append({"neigh": hbuf, "selfv": sbuf, **consts})
    return in_maps


def unpack_output(results):
    """results[k]['out'] [128, NS*D] bf16 -> full [B, D] f32."""
    out = np.empty((B, D), dtype=np.float32)
    for k in range(NCORES):
        res = np.asarray(results[k]["out"], dtype=np.float32)
        full = res.reshape(128, NS, D).transpose(1, 0, 2).reshape(NODES_PAD, D)
        out[k * NODES_PER_CORE : (k + 1) * NODES_PER_CORE] = full[:NODES_PER_CORE]
    return out


def kernel(self_vecs, neigh_vecs, W, a_self, a_neigh):
    from concourse.bass_utils import run_bass_kernel_spmd

    nc = _get_program()
    in_maps = build_in_maps(self_vecs, neigh_vecs, W, a_self, a_neigh)
    res = run_bass_kernel_spmd(nc, in_maps, list(range(NCORES)))
    return unpack_output(res.results)


# revision 59
# speedup vs baseline: 1.1247x; 1.1247x over previous
"""GAT-style attention head (gnn_message_passing) on 8 Trainium2 NeuronCores.

Math (per node b with N=32 neighbors, D=U=128):
    c_n = W @ a_neigh ; c_s = W @ a_self                  (tiny, host)
    nf1[b,n] = neigh[b,n,:] . c_n
    sf1[b]   = self[b,:] . c_s
    e        = exp(lrelu(sf1[b] + nf1[b,n]))
    agg[b,:] = sum_n e[b,n] * neigh[b,n,:]
    Z[b]     = sum_n e[b,n]
    out[b,:] = lrelu((agg[b,:] @ W) / Z[b])

Device-side trick set (vs the f32 baseline, 1.28ms -> ~0.19ms):
  * bf16 inputs in HBM (tolerance is 2e-2): halves HBM traffic -> DMA
    roofline halves to ~155us/core.
  * Host pre-scales neigh rows by c_n elementwise and self rows by c_s, and
    the device projection uses W' = W / c_n[:, None].  Then
      nf1 = row-sum(neigh'),  sf1 = row-sum(self'),  agg' @ W' == agg @ W
    exactly, so the big per-supertile elementwise multiply disappears.
  * Host repacks neigh into supertile-major order so each supertile load is
    one DMA of 128 partitions x 8KB contiguous (line rate).
  * nf1 row-sums as a binary tree of bf16 tensor_tensor adds (these pack at
    2 elem/cycle on the DVE; tensor_reduce does not pack) down to 8 partials,
    then one small f32 tensor_reduce.
  * Supertile loads alternate between the two HWDGE rings (nc.sync /
    nc.scalar) - each ring serializes its own DMAs, alternating keeps two
    loads in flight and the SDMA engines saturated.
  * sf1 broadcast to (row, tile) layout via a tiny mask matmul on the PE
    (replaces a DRAM roundtrip).
  * e4 block-diagonal built by one GpSimd tensor_tensor with a stride-0
    broadcast AP (replaces 4 small scatter DMAs).
  * lrelu via ACT Prelu (same activation table as Exp/Copy -> no table
    reloads); final scale-by-1/Z fused into the last Prelu.
  * Epilogue (1/Z, projection, output write) software-pipelined one
    iteration late so the in-order DVE queue never waits on the PE's Z.

Sharding: batch dim across 8 cores (6250 nodes/core, padded to 6272 = 49
supertiles of 128 nodes; a supertile = 32 tiles of 128 (b,n)-rows = 4 nodes).
"""

import numpy as np
import ml_dtypes

B, N, D = 50000, 32, 128
NCORES = 8
NODES_PER_CORE = B // NCORES            # 6250
SUPER = 128                              # nodes per supertile
NS = (NODES_PER_CORE + SUPER - 1) // SUPER   # 49 supertiles
NODES_PAD = NS * SUPER                   # 6272
ROWS_SUPER = SUPER * N                   # 4096 (b,n)-rows per supertile
NTILES = ROWS_SUPER // 128               # 32 tiles per supertile
ALPHA = 0.2
BF16 = ml_dtypes.bfloat16


def build_program(ns=NS):
    from concourse import mybir
    from concourse.bacc import Bacc
    from concourse.bass import ds
    from concourse.tile import TileContext

    f32 = mybir.dt.float32
    bf16 = mybir.dt.bfloat16
    nc = Bacc()

    neigh = nc.declare_dram_parameter("neigh", [ns * 128, NTILES * D], bf16, isOutput=False)
    selfv = nc.declare_dram_parameter("selfv", [128, ns * D], bf16, isOutput=False)
    wp_in = nc.declare_dram_parameter("wprime", [D, D], bf16, isOutput=False)
    selmask_in = nc.declare_dram_parameter("selmask", [128, 128], bf16, isOutput=False)
    mask4_in = nc.declare_dram_parameter("mask4", [128, NTILES], bf16, isOutput=False)
    prep_in = nc.declare_dram_parameter("prep", [128, 128], bf16, isOutput=False)
    ones1_in = nc.declare_dram_parameter("ones1", [128, 1], bf16, isOutput=False)
    out_d = nc.declare_dram_parameter("out", [128, ns * D], bf16, isOutput=True)

    add = mybir.AluOpType.add
    mult = mybir.AluOpType.mult
    AF = mybir.ActivationFunctionType

    with TileContext(nc) as tc:
        with (
            tc.tile_pool(name="consts", bufs=1) as cpool,
            tc.tile_pool(name="x", bufs=8) as xpool,
            tc.tile_pool(name="work", bufs=4) as wpool,
            tc.tile_pool(name="small", bufs=6) as spool,
            tc.tile_pool(name="psA", bufs=2, space="PSUM") as psA,
            tc.tile_pool(name="psB", bufs=2, space="PSUM") as psB,
            tc.tile_pool(name="psZ", bufs=2, space="PSUM") as psZ,
            tc.tile_pool(name="psS", bufs=2, space="PSUM") as psS,
        ):
            w_sb = cpool.tile([128, D], bf16, tag="w")
            selmask = cpool.tile([128, 128], bf16, tag="selmask")
            mask4 = cpool.tile([128, NTILES], bf16, tag="mask4")
            prep = cpool.tile([128, 128], bf16, tag="prep")
            ones1 = cpool.tile([128, 1], bf16, tag="ones1")
            self_sb = cpool.tile([128, ns * D], bf16, tag="selfsb")
            out_acc = cpool.tile([128, ns * D], bf16, tag="outacc")
            # selfv leads the sync ring (sf1 chain needs it first); the
            # small consts ride the scalar ring so x[1] still lands early.
            nc.sync.dma_start(out=self_sb, in_=selfv[:, :])
            nc.scalar.dma_start(out=w_sb, in_=wp_in[:, :])
            nc.scalar.dma_start(out=selmask, in_=selmask_in[:, :])
            nc.scalar.dma_start(out=mask4, in_=mask4_in[:, :])
            nc.scalar.dma_start(out=prep, in_=prep_in[:, :])
            nc.scalar.dma_start(out=ones1, in_=ones1_in[:, :])

            # sf1 for all supertiles: two bf16 tree levels (2 elem/cycle)
            # then one f32 reduce: [128, ns]
            sf1_all = cpool.tile([128, ns], f32, tag="sf1all")
            with nc.allow_low_precision("bf16 tree partials, f32 final"):
                sft1 = cpool.tile([128, ns * 64], bf16, tag="sft1")
                sview = self_sb.rearrange("p (s2 d) -> p s2 d", s2=ns)
                nc.vector.tensor_tensor(
                    sft1.rearrange("p (s2 d) -> p s2 d", s2=ns),
                    sview[:, :, ds(0, 64)], sview[:, :, ds(64, 64)], op=add,
                )
                sft2 = cpool.tile([128, ns * 32], bf16, tag="sft2")
                t1v = sft1.rearrange("p (s2 d) -> p s2 d", s2=ns)
                nc.vector.tensor_tensor(
                    sft2.rearrange("p (s2 d) -> p s2 d", s2=ns),
                    t1v[:, :, ds(0, 32)], t1v[:, :, ds(32, 32)], op=add,
                )
            nc.vector.tensor_reduce(
                sf1_all,
                sft2.rearrange("p (s2 d) -> p s2 d", s2=ns),
                mybir.AxisListType.X,
                add,
            )

            prev = None
            for s in range(ns):
                # ---- supertile load: [128 rows, 32 tiles * 128 d], contiguous ----
                x = xpool.tile([128, NTILES * D], bf16, tag="x")
                dma_eng = nc.sync if s % 2 == 0 else nc.scalar
                dma_eng.dma_start(out=x, in_=neigh[ds(128 * s, 128), :])

                # ---- nf1 for all 32 tiles: binary-tree halving adds in bf16
                # (tensor_tensor packs at 2 elem/cycle; tensor_reduce does
                # not), down to 4 partials per tile, then a small f32 reduce.
                # Tree depth 5 keeps rounding error ~sqrt(log d) not sqrt(d).
                lvl_in = x
                w = D
                with nc.allow_low_precision("bf16 tree partials, f32 final"):
                    while w > 8:
                        h = w // 2
                        nxt = wpool.tile([128, NTILES * h], bf16, tag=f"tree{h}")
                        nc.vector.tensor_tensor(
                            nxt.rearrange("p (i d) -> p i d", i=NTILES),
                            lvl_in.rearrange("p (i d) -> p i d", i=NTILES)[
                                :, :, ds(0, h)
                            ],
                            lvl_in.rearrange("p (i d) -> p i d", i=NTILES)[
                                :, :, ds(h, h)
                            ],
                            op=add,
                        )
                        lvl_in = nxt
                        w = h
                F1 = spool.tile([128, NTILES], f32, tag="F1")
                nc.vector.tensor_reduce(
                    F1,
                    lvl_in.rearrange("p (i m) -> p i m", m=8),
                    mybir.AxisListType.X,
                    add,
                )

                # ---- sf1e[p=(g,q), i] = sf1[4i+g] via mask matmul ----
                rhsv = spool.tile([128, NTILES], bf16, tag="rhsv")
                nc.scalar.activation(
                    rhsv, mask4, AF.Copy, scale=sf1_all[:, ds(s, 1)]
                )
                sf1e_ps = psS.tile([128, NTILES], f32, tag="sf1e")
                nc.tensor.matmul(sf1e_ps, selmask, rhsv, start=True, stop=True)

                # ---- logits -> lrelu (ACT Prelu) -> e (ACT Exp, bf16) ----
                lk = spool.tile([128, NTILES], f32, tag="lk")
                nc.vector.tensor_tensor(lk, F1, sf1e_ps, op=add)
                lr = spool.tile([128, NTILES], f32, tag="lr")
                nc.scalar.activation(lr, lk, AF.Prelu, alpha=ALPHA)
                e_t = spool.tile([128, NTILES], bf16, tag="e")
                nc.scalar.activation(e_t, lr, AF.Exp)

                # ---- e4 block-diagonal: prep * broadcast(e_t) in one DVE op ----
                e4 = wpool.tile([128, NTILES * 4], bf16, tag="e4")
                e_bc = e_t[:, :].rearrange("p (i o) -> p i o", o=1).broadcast_to(
                    (128, NTILES, 4)
                )
                nc.gpsimd.tensor_tensor(
                    e4.rearrange("p (i g) -> p i g", g=4),
                    prep.rearrange("p (i g) -> p i g", g=4),
                    e_bc,
                    op=mult,
                )

                # ---- aggregation and Z on the PE (z first: it unblocks the
                # delayed epilogue sooner) ----
                z_ps = psZ.tile([128, 1], f32, tag="z")
                nc.tensor.matmul(z_ps, e4, ones1, start=True, stop=True)
                aggT_ps = psA.tile([128, 128], f32, tag="aggT")
                for i in range(NTILES):
                    nc.tensor.matmul(
                        aggT_ps[:, ds(4 * i, 4)],
                        x[:, ds(D * i, D)],
                        e4[:, ds(4 * i, 4)],
                        start=True, stop=True,
                    )

                # ---- epilogue for the PREVIOUS supertile (software pipelining:
                # recip depends on the end of this iteration's serial chain, so
                # running it one iteration late keeps the in-order DVE queue
                # bubble-free) ----
                if prev is not None:
                    p_s, p_z, p_agg = prev
                    zinv = spool.tile([128, 1], f32, tag="zinv")
                    nc.vector.reciprocal(zinv, p_z)
                    aggT_sb = wpool.tile([128, 128], bf16, tag="aggTsb")
                    nc.scalar.copy(aggT_sb, p_agg)
                    out_ps = psB.tile([128, 128], f32, tag="outps")
                    nc.tensor.matmul(out_ps, aggT_sb, w_sb, start=True, stop=True)
                    nc.scalar.activation(
                        out_acc[:, ds(D * p_s, D)], out_ps, AF.Prelu,
                        scale=zinv[:, :], alpha=ALPHA,
                    )
                prev = (s, z_ps, aggT_ps)

            # drain the last supertile's epilogue
            p_s, p_z, p_agg = prev
            zinv = spool.tile([128, 1], f32, tag="zinv")
            nc.vector.reciprocal(zinv, p_z)
            aggT_sb = wpool.tile([128, 128], bf16, tag="aggTsb")
            nc.scalar.copy(aggT_sb, p_agg)
            out_ps = psB.tile([128, 128], f32, tag="outps")
            nc.tensor.matmul(out_ps, aggT_sb, w_sb, start=True, stop=True)
            nc.scalar.activation(
                out_acc[:, ds(D * p_s, D)], out_ps, AF.Prelu,
                scale=zinv[:, :], alpha=ALPHA,
            )

            nc.sync.dma_start(out=out_d[:, :], in_=out_acc)

    nc.compile()
    return nc


_CACHE = {}


def _get_program():
    if "nc" not in _CACHE:
        _CACHE["nc"] = build_program()
    return _CACHE["nc"]


def make_consts(W, a_self, a_neigh):
    """Host-side parameter prep: c_n/c_s projections and mask constants."""
    W = np.ascontiguousarray(W, dtype=np.float32)
    c_n = (W @ np.asarray(a_neigh, dtype=np.float32)[:, 0]).astype(np.float32)
    c_s = (W @ np.asarray(a_self, dtype=np.float32)[:, 0]).astype(np.float32)
    # guard against pathological zeros (probability ~0 for random W)
    c_n_safe = np.where(np.abs(c_n) < 1e-30, 1e-30, c_n).astype(np.float32)
    wprime = (W / c_n_safe[:, None]).astype(BF16)

    p = np.arange(128)
    i = np.arange(NTILES)
    selmask = (p[:, None] % 4 == (np.arange(128)[None, :] // 32)).astype(BF16)
    mask4 = (p[:, None] // 4 == i[None, :]).astype(BF16)
    prep = ((p[:, None] // 32) == (np.arange(128)[None, :] % 4)).astype(BF16)
    ones1 = np.ones((128, 1), dtype=BF16)
    consts = {
        "wprime": wprime,
        "selmask": selmask,
        "mask4": mask4,
        "prep": prep,
        "ones1": ones1,
    }
    return consts, c_n, c_s


def build_in_maps(self_vecs, neigh_vecs, W, a_self, a_neigh):
    """Shard + pre-scale + repack all inputs into per-core DRAM images."""
    consts, c_n, c_s = make_consts(W, a_self, a_neigh)

    neigh_vecs = np.asarray(neigh_vecs, dtype=np.float32)
    self_vecs = np.asarray(self_vecs, dtype=np.float32)
    # pre-scale (folded back exactly by W' = W / c_n on-device)
    xs_all = (neigh_vecs * c_n[None, None, :]).astype(BF16)     # [B, N, D]
    ss_all = (self_vecs * c_s[None, :]).astype(BF16)            # [B, D]

    in_maps = []
    for k in range(NCORES):
        lo = k * NODES_PER_CORE
        hi = lo + NODES_PER_CORE
        xs = np.zeros((NODES_PAD, N, D), dtype=BF16)
        xs[:NODES_PER_CORE] = xs_all[lo:hi]
        # rows r = 32*b_local + n ; tile i = rows 128i..128i+127 ; partition p
        # hbuf[s, p, i, d] = xs_rows[s, 128*i + p, d]
        hbuf = (
            xs.reshape(NS, SUPER * N, D)
            .reshape(NS, NTILES, 128, D)
            .transpose(0, 2, 1, 3)
            .reshape(NS * 128, NTILES * D)
        )
        hbuf = np.ascontiguousarray(hbuf)

        ss = np.zeros((NODES_PAD, D), dtype=BF16)
        ss[:NODES_PER_CORE] = ss_all[lo:hi]
        sbuf = np.ascontiguousarray(
            ss.reshape(NS, 128, D).transpose(1, 0, 2).reshape(128, NS * D)
        )
        in_maps.append({"neigh": hbuf, "selfv": sbuf, **consts})
    return in_maps


def unpack_output(results):
    """results[k]['out'] [128, NS*D] bf16 -> full [B, D] f32."""
    out = np.empty((B, D), dtype=np.float32)
    for k in range(NCORES):
        res = np.asarray(results[k]["out"], dtype=np.float32)
        full = res.reshape(128, NS, D).transpose(1, 0, 2).reshape(NODES_PAD, D)
        out[k * NODES_PER_CORE : (k + 1) * NODES_PER_CORE] = full[:NODES_PER_CORE]
    return out


def kernel(self_vecs, neigh_vecs, W, a_self, a_neigh):
    from concourse.bass_utils import run_bass_kernel_spmd

    nc = _get_program()
    in_maps = build_in_maps(self_vecs, neigh_vecs, W, a_self, a_neigh)
    res = run_bass_kernel_spmd(nc, in_maps, list(range(NCORES)))
    return unpack_output(res.results)


# revision 60
# speedup vs baseline: 1.1429x; 1.0162x over previous
"""GAT-style attention head (gnn_message_passing) on 8 Trainium2 NeuronCores.

Math (per node b with N=32 neighbors, D=U=128):
    c_n = W @ a_neigh ; c_s = W @ a_self                  (tiny, host)
    nf1[b,n] = neigh[b,n,:] . c_n
    sf1[b]   = self[b,:] . c_s
    e        = exp(lrelu(sf1[b] + nf1[b,n]))
    agg[b,:] = sum_n e[b,n] * neigh[b,n,:]
    Z[b]     = sum_n e[b,n]
    out[b,:] = lrelu((agg[b,:] @ W) / Z[b])

Device-side trick set (vs the f32 baseline, 1.28ms -> ~0.19ms):
  * bf16 inputs in HBM (tolerance is 2e-2): halves HBM traffic -> DMA
    roofline halves to ~155us/core.
  * Host pre-scales neigh rows by c_n elementwise and self rows by c_s, and
    the device projection uses W' = W / c_n[:, None].  Then
      nf1 = row-sum(neigh'),  sf1 = row-sum(self'),  agg' @ W' == agg @ W
    exactly, so the big per-supertile elementwise multiply disappears.
  * Host repacks neigh into supertile-major order so each supertile load is
    one DMA of 128 partitions x 8KB contiguous (line rate).
  * nf1 row-sums as a binary tree of bf16 tensor_tensor adds (these pack at
    2 elem/cycle on the DVE; tensor_reduce does not pack) down to 8 partials,
    then one small f32 tensor_reduce.
  * Supertile loads alternate between the two HWDGE rings (nc.sync /
    nc.scalar) - each ring serializes its own DMAs, alternating keeps two
    loads in flight and the SDMA engines saturated.
  * sf1 broadcast to (row, tile) layout via a tiny mask matmul on the PE
    (replaces a DRAM roundtrip).
  * e4 block-diagonal built by one GpSimd tensor_tensor with a stride-0
    broadcast AP (replaces 4 small scatter DMAs).
  * lrelu via ACT Prelu (same activation table as Exp/Copy -> no table
    reloads); final scale-by-1/Z fused into the last Prelu.
  * Epilogue (1/Z, projection, output write) software-pipelined one
    iteration late so the in-order DVE queue never waits on the PE's Z.

Sharding: batch dim across 8 cores (6250 nodes/core, padded to 6272 = 49
supertiles of 128 nodes; a supertile = 32 tiles of 128 (b,n)-rows = 4 nodes).
"""

import numpy as np
import ml_dtypes

B, N, D = 50000, 32, 128
NCORES = 8
NODES_PER_CORE = B // NCORES            # 6250
SUPER = 128                              # nodes per supertile
NS = (NODES_PER_CORE + SUPER - 1) // SUPER   # 49 supertiles
NODES_PAD = NS * SUPER                   # 6272
ROWS_SUPER = SUPER * N                   # 4096 (b,n)-rows per supertile
NTILES = ROWS_SUPER // 128               # 32 tiles per supertile
ALPHA = 0.2
BF16 = ml_dtypes.bfloat16


def build_program(ns=NS):
    from concourse import mybir
    from concourse.bacc import Bacc
    from concourse.bass import ds
    from concourse.tile import TileContext

    f32 = mybir.dt.float32
    bf16 = mybir.dt.bfloat16
    nc = Bacc()

    neigh = nc.declare_dram_parameter("neigh", [ns * 128, NTILES * D], bf16, isOutput=False)
    selfv = nc.declare_dram_parameter("selfv", [128, ns * D], bf16, isOutput=False)
    wp_in = nc.declare_dram_parameter("wprime", [D, D], bf16, isOutput=False)
    selmask_in = nc.declare_dram_parameter("selmask", [128, 128], bf16, isOutput=False)
    mask4_in = nc.declare_dram_parameter("mask4", [128, NTILES], bf16, isOutput=False)
    prep_in = nc.declare_dram_parameter("prep", [128, 128], bf16, isOutput=False)
    ones1_in = nc.declare_dram_parameter("ones1", [128, 1], bf16, isOutput=False)
    out_d = nc.declare_dram_parameter("out", [128, ns * D], bf16, isOutput=True)

    add = mybir.AluOpType.add
    mult = mybir.AluOpType.mult
    AF = mybir.ActivationFunctionType

    with TileContext(nc) as tc:
        with (
            tc.tile_pool(name="consts", bufs=1) as cpool,
            tc.tile_pool(name="x", bufs=8) as xpool,
            tc.tile_pool(name="work", bufs=4) as wpool,
            tc.tile_pool(name="small", bufs=6) as spool,
            tc.tile_pool(name="psA", bufs=2, space="PSUM") as psA,
            tc.tile_pool(name="psB", bufs=2, space="PSUM") as psB,
            tc.tile_pool(name="psZ", bufs=2, space="PSUM") as psZ,
            tc.tile_pool(name="psS", bufs=2, space="PSUM") as psS,
        ):
            w_sb = cpool.tile([128, D], bf16, tag="w")
            selmask = cpool.tile([128, 128], bf16, tag="selmask")
            mask4 = cpool.tile([128, NTILES], bf16, tag="mask4")
            prep = cpool.tile([128, 128], bf16, tag="prep")
            ones1 = cpool.tile([128, 1], bf16, tag="ones1")
            self_sb = cpool.tile([128, ns * D], bf16, tag="selfsb")
            out_acc = cpool.tile([128, ns * D], bf16, tag="outacc")
            # selfv leads the sync ring; x[0] leads the scalar ring so
            # tree[0] can start immediately; consts follow on scalar.
            nc.sync.dma_start(out=self_sb, in_=selfv[:, :])
            x0 = xpool.tile([128, NTILES * D], bf16, tag="x")
            nc.scalar.dma_start(out=x0, in_=neigh[ds(0, 128), :])
            nc.scalar.dma_start(out=w_sb, in_=wp_in[:, :])
            nc.scalar.dma_start(out=selmask, in_=selmask_in[:, :])
            nc.scalar.dma_start(out=mask4, in_=mask4_in[:, :])
            nc.scalar.dma_start(out=prep, in_=prep_in[:, :])
            nc.scalar.dma_start(out=ones1, in_=ones1_in[:, :])

            sf1_all = cpool.tile([128, ns], f32, tag="sf1all")

            prev = None
            for s in range(ns):
                # ---- supertile load: [128 rows, 32 tiles * 128 d], contiguous ----
                if s == 0:
                    x = x0
                else:
                    x = xpool.tile([128, NTILES * D], bf16, tag="x")
                    dma_eng = nc.scalar if s % 2 == 0 else nc.sync
                    dma_eng.dma_start(out=x, in_=neigh[ds(128 * s, 128), :])

                # ---- nf1 for all 32 tiles: binary-tree halving adds in bf16
                # (tensor_tensor packs at 2 elem/cycle; tensor_reduce does
                # not), down to 4 partials per tile, then a small f32 reduce.
                # Tree depth 5 keeps rounding error ~sqrt(log d) not sqrt(d).
                lvl_in = x
                w = D
                with nc.allow_low_precision("bf16 tree partials, f32 final"):
                    while w > 8:
                        h = w // 2
                        nxt = wpool.tile([128, NTILES * h], bf16, tag=f"tree{h}")
                        nc.vector.tensor_tensor(
                            nxt.rearrange("p (i d) -> p i d", i=NTILES),
                            lvl_in.rearrange("p (i d) -> p i d", i=NTILES)[
                                :, :, ds(0, h)
                            ],
                            lvl_in.rearrange("p (i d) -> p i d", i=NTILES)[
                                :, :, ds(h, h)
                            ],
                            op=add,
                        )
                        lvl_in = nxt
                        w = h
                F1 = spool.tile([128, NTILES], f32, tag="F1")
                nc.vector.tensor_reduce(
                    F1,
                    lvl_in.rearrange("p (i m) -> p i m", m=8),
                    mybir.AxisListType.X,
                    add,
                )

                if s == 0:
                    # sf1 for all supertiles: two bf16 tree levels then one
                    # f32 reduce, emitted after tree[0] so the first tree
                    # leads the in-order DVE queue.
                    with nc.allow_low_precision("bf16 tree partials, f32 final"):
                        sft1 = cpool.tile([128, ns * 64], bf16, tag="sft1")
                        sview = self_sb.rearrange("p (s2 d) -> p s2 d", s2=ns)
                        nc.vector.tensor_tensor(
                            sft1.rearrange("p (s2 d) -> p s2 d", s2=ns),
                            sview[:, :, ds(0, 64)], sview[:, :, ds(64, 64)],
                            op=add,
                        )
                        sft2 = cpool.tile([128, ns * 32], bf16, tag="sft2")
                        t1v = sft1.rearrange("p (s2 d) -> p s2 d", s2=ns)
                        nc.vector.tensor_tensor(
                            sft2.rearrange("p (s2 d) -> p s2 d", s2=ns),
                            t1v[:, :, ds(0, 32)], t1v[:, :, ds(32, 32)], op=add,
                        )
                    nc.vector.tensor_reduce(
                        sf1_all,
                        sft2.rearrange("p (s2 d) -> p s2 d", s2=ns),
                        mybir.AxisListType.X,
                        add,
                    )

                # ---- sf1e[p=(g,q), i] = sf1[4i+g] via mask matmul ----
                rhsv = spool.tile([128, NTILES], bf16, tag="rhsv")
                nc.scalar.activation(
                    rhsv, mask4, AF.Copy, scale=sf1_all[:, ds(s, 1)]
                )
                sf1e_ps = psS.tile([128, NTILES], f32, tag="sf1e")
                nc.tensor.matmul(sf1e_ps, selmask, rhsv, start=True, stop=True)

                # ---- logits -> lrelu (ACT Prelu) -> e (ACT Exp, bf16) ----
                lk = spool.tile([128, NTILES], f32, tag="lk")
                nc.vector.tensor_tensor(lk, F1, sf1e_ps, op=add)
                lr = spool.tile([128, NTILES], f32, tag="lr")
                nc.scalar.activation(lr, lk, AF.Prelu, alpha=ALPHA)
                e_t = spool.tile([128, NTILES], bf16, tag="e")
                nc.scalar.activation(e_t, lr, AF.Exp)

                # ---- e4 block-diagonal: prep * broadcast(e_t) in one DVE op ----
                e4 = wpool.tile([128, NTILES * 4], bf16, tag="e4")
                e_bc = e_t[:, :].rearrange("p (i o) -> p i o", o=1).broadcast_to(
                    (128, NTILES, 4)
                )
                nc.gpsimd.tensor_tensor(
                    e4.rearrange("p (i g) -> p i g", g=4),
                    prep.rearrange("p (i g) -> p i g", g=4),
                    e_bc,
                    op=mult,
                )

                # ---- aggregation and Z on the PE (z first: it unblocks the
                # delayed epilogue sooner) ----
                z_ps = psZ.tile([128, 1], f32, tag="z")
                nc.tensor.matmul(z_ps, e4, ones1, start=True, stop=True)
                aggT_ps = psA.tile([128, 128], f32, tag="aggT")
                for i in range(NTILES):
                    nc.tensor.matmul(
                        aggT_ps[:, ds(4 * i, 4)],
                        x[:, ds(D * i, D)],
                        e4[:, ds(4 * i, 4)],
                        start=True, stop=True,
                    )

                # ---- epilogue for the PREVIOUS supertile (software pipelining:
                # recip depends on the end of this iteration's serial chain, so
                # running it one iteration late keeps the in-order DVE queue
                # bubble-free) ----
                if prev is not None:
                    p_s, p_z, p_agg = prev
                    zinv = spool.tile([128, 1], f32, tag="zinv")
                    nc.vector.reciprocal(zinv, p_z)
                    aggT_sb = wpool.tile([128, 128], bf16, tag="aggTsb")
                    nc.scalar.copy(aggT_sb, p_agg)
                    out_ps = psB.tile([128, 128], f32, tag="outps")
                    nc.tensor.matmul(out_ps, aggT_sb, w_sb, start=True, stop=True)
                    nc.scalar.activation(
                        out_acc[:, ds(D * p_s, D)], out_ps, AF.Prelu,
                        scale=zinv[:, :], alpha=ALPHA,
                    )
                prev = (s, z_ps, aggT_ps)

            # drain the last supertile's epilogue
            p_s, p_z, p_agg = prev
            zinv = spool.tile([128, 1], f32, tag="zinv")
            nc.vector.reciprocal(zinv, p_z)
            aggT_sb = wpool.tile([128, 128], bf16, tag="aggTsb")
            nc.scalar.copy(aggT_sb, p_agg)
            out_ps = psB.tile([128, 128], f32, tag="outps")
            nc.tensor.matmul(out_ps, aggT_sb, w_sb, start=True, stop=True)
            nc.scalar.activation(
                out_acc[:, ds(D * p_s, D)], out_ps, AF.Prelu,
                scale=zinv[:, :], alpha=ALPHA,
            )

            nc.sync.dma_start(out=out_d[:, :], in_=out_acc)

    nc.compile()
    return nc


_CACHE = {}


def _get_program():
    if "nc" not in _CACHE:
        _CACHE["nc"] = build_program()
    return _CACHE["nc"]


def make_consts(W, a_self, a_neigh):
    """Host-side parameter prep: c_n/c_s projections and mask constants."""
    W = np.ascontiguousarray(W, dtype=np.float32)
    c_n = (W @ np.asarray(a_neigh, dtype=np.float32)[:, 0]).astype(np.float32)
    c_s = (W @ np.asarray(a_self, dtype=np.float32)[:, 0]).astype(np.float32)
    # guard against pathological zeros (probability ~0 for random W)
    c_n_safe = np.where(np.abs(c_n) < 1e-30, 1e-30, c_n).astype(np.float32)
    wprime = (W / c_n_safe[:, None]).astype(BF16)

    p = np.arange(128)
    i = np.arange(NTILES)
    selmask = (p[:, None] % 4 == (np.arange(128)[None, :] // 32)).astype(BF16)
    mask4 = (p[:, None] // 4 == i[None, :]).astype(BF16)
    prep = ((p[:, None] // 32) == (np.arange(128)[None, :] % 4)).astype(BF16)
    ones1 = np.ones((128, 1), dtype=BF16)
    consts = {
        "wprime": wprime,
        "selmask": selmask,
        "mask4": mask4,
        "prep": prep,
        "ones1": ones1,
    }
    return consts, c_n, c_s


def build_in_maps(self_vecs, neigh_vecs, W, a_self, a_neigh):
    """Shard + pre-scale + repack all inputs into per-core DRAM images."""
    consts, c_n, c_s = make_consts(W, a_self, a_neigh)

    neigh_vecs = np.asarray(neigh_vecs, dtype=np.float32)
    self_vecs = np.asarray(self_vecs, dtype=np.float32)
    # pre-scale (folded back exactly by W' = W / c_n on-device)
    xs_all = (neigh_vecs * c_n[None, None, :]).astype(BF16)     # [B, N, D]
    ss_all = (self_vecs * c_s[None, :]).astype(BF16)            # [B, D]

    in_maps = []
    for k in range(NCORES):
        lo = k * NODES_PER_CORE
        hi = lo + NODES_PER_CORE
        xs = np.zeros((NODES_PAD, N, D), dtype=BF16)
        xs[:NODES_PER_CORE] = xs_all[lo:hi]
        # rows r = 32*b_local + n ; tile i = rows 128i..128i+127 ; partition p
        # hbuf[s, p, i, d] = xs_rows[s, 128*i + p, d]
        hbuf = (
            xs.reshape(NS, SUPER * N, D)
            .reshape(NS, NTILES, 128, D)
            .transpose(0, 2, 1, 3)
            .reshape(NS * 128, NTILES * D)
        )
        hbuf = np.ascontiguousarray(hbuf)

        ss = np.zeros((NODES_PAD, D), dtype=BF16)
        ss[:NODES_PER_CORE] = ss_all[lo:hi]
        sbuf = np.ascontiguousarray(
            ss.reshape(NS, 128, D).transpose(1, 0, 2).reshape(128, NS * D)
        )
        in_maps.append({"neigh": hbuf, "selfv": sbuf, **consts})
    return in_maps


def unpack_output(results):
    """results[k]['out'] [128, NS*D] bf16 -> full [B, D] f32."""
    out = np.empty((B, D), dtype=np.float32)
    for k in range(NCORES):
        res = np.asarray(results[k]["out"], dtype=np.float32)
        full = res.reshape(128, NS, D).transpose(1, 0, 2).reshape(NODES_PAD, D)
        out[k * NODES_PER_CORE : (k + 1) * NODES_PER_CORE] = full[:NODES_PER_CORE]
    return out


def kernel(self_vecs, neigh_vecs, W, a_self, a_neigh):
    from concourse.bass_utils import run_bass_kernel_spmd

    nc = _get_program()
    in_maps = build_in_maps(self_vecs, neigh_vecs, W, a_self, a_neigh)
    res = run_bass_kernel_spmd(nc, in_maps, list(range(NCORES)))
    return unpack_output(res.results)
